# revision 2
# baseline (speedup 1.0000x reference)
"""CoxLoss (nn_CoxLoss) Trainium2 kernel v2: two-level histogram/CDF, 8-way
SPMD, AllGather-based table exchange.

risk_i = sum_{b<=a_i, t<=u_i} G2[t,b]  (2D prefix of folded 2nd differences)

  a_i = floor(s_i*B1) (level-1 bin), u_i = (s_i*B1*B2) mod B2
  Msuf[b,t] = sum_j w_j [a_j>=b][u_j>=t]  (PE matmul over j staircases,
              sharded over cores, AllGather + local reduce)
  G2 = 2nd differences over b of the t-diff of Msuf + suffix-histogram fold

Key = 2^10 bins: loss rel err ~4.9e-4 (same-key distinct-value pairs),
inside the 2e-2 gate with 40x margin. AllGather (15us fixed + 32KB) replaces
AllReduce (28.1us fixed min); the local reduce and all i-side staircases
hide under the collective.
"""
import numpy as np
import concourse.bass as bass
import concourse.mybir as mybir
from concourse.tile import TileContext
from concourse.bass_utils import run_bass_kernel_spmd

F32 = mybir.dt.float32
BF16 = mybir.dt.bfloat16
I32 = mybir.dt.int32
AF = mybir.ActivationFunctionType
ALU = mybir.AluOpType

N = 16384
P = 128
NCORES = 8
MY = N // NCORES          # 2048 rows per core
JCH = MY // P             # 16 j-chunks per core
B1 = 32                   # level-1 (value) bins: b
B2 = 32                   # level-2 bins: t
SCALE2 = float(B1 * B2)   # 2^10
QF = 512                  # i-tile width
NQ = MY // QF             # 4 i-tiles
NFOLD = P // B1           # 4 table blocks folded per matmul

# in_pack columns
C_S = 0              # s chunk-major [128,16]
C_TH = JCH           # theta
C_CEN = 2 * JCH      # censor
C_IB = 3 * JCH       # ib col (1): p/B1
C_IT = C_IB + 1      # it col (1): p
C_IT2 = C_IT + 1     # it2 col (1): p %% B2 for p<2*B2 else big
C_IDS = C_IT2 + 1    # identS2 [128,2*B1]: S_h[p,b]=[b==(2p+h)%B1]
C_IST = C_IDS + 2 * B1   # Istack [128,2*B2]: rows 0..B2-1: [o%%B2==p]
PACKW = C_IST + 2 * B2


def legalize_waits(nc, max_waits=1):
    """Insert same-engine Drains carrying excess sync waits immediately
    before each offending instruction (walrus accepts ~1 wait/instr)."""
    fn = nc.m.functions[0]
    for blk in fn.blocks:
        insts = blk.instructions
        out_list = []
        changed = False
        for ins in insts:
            si = ins.sync_info
            if si is not None and len(si.on_wait) > max_waits:
                waits = list(si.on_wait)
                keep = waits[:max_waits]
                for k, w in enumerate(waits[max_waits:]):
                    d = mybir.InstDrain(name=f"{ins.name}-w{k}", ins=[], outs=[])
                    d.engine = ins.engine
                    d.sync_info = mybir.SyncInfo(on_wait=[w], on_update=[])
                    out_list.append(d)
                si.on_wait = keep
                ins.sync_info = si
                changed = True
            out_list.append(ins)
        if changed:
            blk.instructions = out_list


def build():
    nc = bass.Bass()
    in_pack = nc.dram_tensor("in_pack", [P, PACKW], F32, kind="ExternalInput")
    in_row = nc.dram_tensor("in_row", [1, MY], F32, kind="ExternalInput")
    in_iota = nc.dram_tensor("in_iota", [1, B1 + B2], F32, kind="ExternalInput")
    out = nc.dram_tensor("partial", [1, 1], F32, kind="ExternalOutput")

    msuf_dram = nc.dram_tensor("msuf_dram", [B1, B2], F32)
    msuf_sh = nc.dram_tensor("msuf_sh", [P, 2 * B2], F32,
                             addr_space="Shared")

    with TileContext(nc) as tc:
        with (
            tc.tile_pool(name="const", bufs=1) as cpool,
            tc.tile_pool(name="jstair", bufs=6) as jpool,
            tc.tile_pool(name="istair", bufs=1) as ipool,
            tc.tile_pool(name="small", bufs=1) as spool,
            tc.tile_pool(name="prod", bufs=2) as prpool,
            tc.tile_pool(name="pms", bufs=1, space="PSUM") as pms,
            tc.tile_pool(name="pyt", bufs=1, space="PSUM") as pyt,
            tc.tile_pool(name="pr1", bufs=2, space="PSUM") as pr1,
            tc.tile_pool(name="pfin", bufs=1, space="PSUM") as pfin,
        ):
            # ---------------- input DMAs: pack on Pool; iota+s_rep on SP.
            # (HWDGE dma_start occupies the issuing engine; keep ACT free.)
            pack = cpool.tile([P, PACKW], F32)
            nc.gpsimd.dma_start(out=pack, in_=in_pack[:, :])
            iota2 = cpool.tile([P, B1 + B2], F32)
            nc.sync.dma_start(out=iota2,
                              in_=in_iota[:, :].to_broadcast([P, B1 + B2]))
            s_rep = cpool.tile([P, MY], F32)
            for q in range(NQ):
                nc.sync.dma_start(
                    out=s_rep[:, q * QF:(q + 1) * QF],
                    in_=in_row[:, q * QF:(q + 1) * QF].to_broadcast([P, QF]))

            s_cols = pack[:, C_S:C_S + JCH]
            th_cols = pack[:, C_TH:C_TH + JCH]
            cen_cols = pack[:, C_CEN:C_CEN + JCH]
            ib_col = pack[:, C_IB:C_IB + 1]
            it_col = pack[:, C_IT:C_IT + 1]
            it2_col = pack[:, C_IT2:C_IT2 + 1]
            identS2 = pack[:, C_IDS:C_IDS + 2 * B1]
            istack = pack[:, C_IST:C_IST + 2 * B2]
            iotaB = iota2[:, 0:B1]
            iotaT = iota2[:, B1:B1 + B2]

            ones_col = cpool.tile([P, 1], F32)
            nc.vector.memset(ones_col, 1.0)

            # ---------------- ACT: preload Sigmoid table on dummy data,
            # then w as soon as pack lands. Ln loads once at lnw and stays
            # resident for the epilogue lnr.
            act_t = cpool.tile([P, 2], F32)
            nc.scalar.activation(out=act_t[:, 0:1], in_=ones_col,
                                 func=AF.Sigmoid)
            w_col = cpool.tile([P, JCH], F32)
            nc.scalar.activation(out=w_col, in_=th_cols, func=AF.Sigmoid)
            lnw = cpool.tile([P, JCH], F32)
            nc.scalar.activation(out=lnw, in_=w_col, func=AF.Ln)

            # DVE touch ops (establish first-use order)
            dve_t = cpool.tile([P, 4], F32)
            nc.vector.tensor_copy(dve_t[:, 0:1], pack[:, 0:1])
            nc.vector.tensor_copy(dve_t[:, 1:2], iota2[:, 0:1])

            # ---------------- v-space sub-bin coord: v = s - floor(s*B1)/B1
            # ([u >= t] == [v >= t/SCALE2]); floor via RNE(x-0.5) i32 convert
            def emit_v(dst, src_ap, wt, it_, ft):
                nc.vector.tensor_scalar(out=wt, in0=src_ap,
                                        scalar1=float(B1), scalar2=0.5,
                                        op0=ALU.mult, op1=ALU.subtract)
                nc.vector.tensor_copy(it_, wt)     # f32 -> i32 (RNE)
                nc.vector.tensor_copy(ft, it_)     # i32 -> f32
                nc.vector.scalar_tensor_tensor(out=dst, in0=ft,
                                               scalar=-1.0 / float(B1),
                                               in1=src_ap,
                                               op0=ALU.mult, op1=ALU.add)

            # j staircases: all daw first (no w/u dep), then v chain, then dcw
            msuf_ps = pms.tile([B1, B2], F32, tag="msuf")
            daws = []
            for jc in range(JCH):
                daw = jpool.tile([P, B1], F32, tag="daw", name=f"daw{jc}")
                nc.vector.tensor_scalar(out=daw, in0=iotaB,
                                        scalar1=s_cols[:, jc:jc + 1],
                                        scalar2=None, op0=ALU.is_le)
                daws.append(daw)

            v_col = cpool.tile([P, JCH], F32)
            uw = cpool.tile([P, JCH], F32)
            ui = cpool.tile([P, JCH], I32)
            uf = cpool.tile([P, JCH], F32)
            emit_v(v_col, s_cols, uw, ui, uf)

            for jc in range(JCH):
                dcw = jpool.tile([P, B2], F32, tag="dcw", name=f"dcw{jc}")
                nc.vector.tensor_scalar(out=dcw, in0=iotaT,
                                        scalar1=v_col[:, jc:jc + 1],
                                        scalar2=w_col[:, jc:jc + 1],
                                        op0=ALU.is_le, op1=ALU.mult)
                nc.tensor.matmul(msuf_ps[:, :], daws[jc], dcw,
                                 start=(jc == 0), stop=(jc == JCH - 1))

            # Msuf -> SBUF (ACT) -> DRAM -> AllGather
            msuf_sb = spool.tile([B1, B2], F32)
            nc.scalar.copy(msuf_sb, msuf_ps[:, :])
            nc.gpsimd.dma_start(out=msuf_dram[:, :], in_=msuf_sb)
            nc.gpsimd.collective_compute(
                "AllGather", ALU.bypass,
                ins=[msuf_dram[:, :]], outs=[msuf_sh[:, :]],
                replica_groups=[list(range(NCORES))])

            # ---------------- i staircases + u_rep (DVE, hidden under AG)
            with tc.tile_wait_until(0.0055):
                uw2 = ipool.tile([P, MY], F32)
                ui2 = ipool.tile([P, MY], I32)
                uf2 = ipool.tile([P, MY], F32)
                v_rep = ipool.tile([P, MY], F32)
                emit_v(v_rep, s_rep, uw2, ui2, uf2)
                dab = ipool.tile([P, MY], F32)
                nc.vector.tensor_scalar(out=dab, in0=s_rep, scalar1=ib_col,
                                        scalar2=None, op0=ALU.is_ge)
                dct2 = ipool.tile([P, MY], BF16)
                nc.vector.tensor_scalar(out=dct2, in0=v_rep, scalar1=it2_col,
                                        scalar2=None, op0=ALU.is_ge)

            # ---------------- gathered tables: one [128, 64] load
            # (row p = gathered rows 2p | 2p+1; col-halves are separate rows)
            mt = spool.tile([P, 2 * B2], F32)
            nc.gpsimd.dma_start(out=mt, in_=msuf_sh[:, :])
            # t-diff within each col-half
            yacc = spool.tile([P, 2 * B2], F32)
            nc.vector.tensor_copy(yacc[:, 0:1], mt[:, 0:1])
            nc.vector.tensor_tensor(out=yacc[:, 1:B2], in0=mt[:, 1:B2],
                                    in1=mt[:, 0:B2 - 1], op=ALU.subtract)
            nc.vector.tensor_copy(yacc[:, B2:B2 + 1], mt[:, B2:B2 + 1])
            nc.vector.tensor_tensor(out=yacc[:, B2 + 1:2 * B2],
                                    in0=mt[:, B2 + 1:2 * B2],
                                    in1=mt[:, B2:2 * B2 - 1], op=ALU.subtract)

            # fold all 8 blocks + transpose in 2 accumulating matmuls:
            # YT[t,b] = sum_h sum_p yacc[p, h*B2+t] * S_h[p, b]
            yt_ps = pyt.tile([B2, B1], F32, tag="ytps")
            nc.tensor.matmul(yt_ps[:, :], yacc[:, 0:B2], identS2[:, 0:B1],
                             start=True, stop=False)
            nc.tensor.matmul(yt_ps[:, :], yacc[:, B2:2 * B2],
                             identS2[:, B1:2 * B1], start=False, stop=True)
            ytp = spool.tile([B2, B1 + 2], F32)
            nc.vector.tensor_copy(ytp[:, 1:B1 + 1], yt_ps[:, :])
            nc.vector.tensor_copy(ytp[:, 0:1], yt_ps[:, 0:1])  # left pad
            nc.vector.memset(ytp[:, B1 + 1:B1 + 2], 0.0)       # right pad

            # G2 = 2*P1 - P2 - P0  [B2, B1]
            g2 = spool.tile([B2, B1], F32)
            nc.vector.scalar_tensor_tensor(out=g2, in0=ytp[:, 1:B1 + 1],
                                           scalar=2.0, in1=ytp[:, 2:B1 + 2],
                                           op0=ALU.mult, op1=ALU.subtract)
            nc.vector.tensor_tensor(out=g2, in0=g2, in1=ytp[:, 0:B1],
                                    op=ALU.subtract)
            # hh fold on t=0 row: H[b] = YT[0, b] = ytp[0, 1+b]
            hh = spool.tile([1, B1], F32)
            nc.vector.tensor_tensor(out=hh[:1, :], in0=ytp[0:1, 2:B1 + 2],
                                    in1=ytp[0:1, 1:B1 + 1], op=ALU.subtract)
            nc.vector.tensor_tensor(out=g2[0:1, :], in0=g2[0:1, :],
                                    in1=hh[:1, :], op=ALU.add)
            nc.vector.tensor_tensor(out=g2[0:1, 0:1], in0=g2[0:1, 0:1],
                                    in1=ytp[0:1, 1:2], op=ALU.add)

            # duplicate g2 rows into [2*B2, B1] (PE), then bf16 hi/lo:
            # rows 0..B2-1 = bf16(g2), rows B2.. = g2 - bf16(g2). One r1
            # matmul per i-tile contracts hi+lo together over 2*B2 rows.
            g2d_ps = pyt.tile([2 * B2, B1], F32, tag="g2dps")
            nc.tensor.matmul(g2d_ps[:, :], istack[0:B2, :], g2,
                             start=True, stop=True)
            g2s = spool.tile([2 * B2, B1], BF16)
            nc.vector.tensor_copy(g2s, g2d_ps[:, :])
            nc.vector.tensor_tensor(out=g2s[B2:2 * B2, :],
                                    in0=g2d_ps[B2:2 * B2, :],
                                    in1=g2s[B2:2 * B2, :], op=ALU.subtract)

            # ---------------- R1 + prod + partition reduce -> risk_pm
            risk_pm = pfin.tile([P, JCH], F32, tag="riskpm")
            for it in range(NQ):
                r1 = pr1.tile([B1, QF], F32, tag="r1", name=f"r1_{it}")
                nc.tensor.matmul(r1[:, :], g2s,
                                 dct2[0:2 * B2, it * QF:(it + 1) * QF],
                                 start=True, stop=True)
                prod = prpool.tile([B1, QF], F32, tag="prod",
                                   name=f"prod{it}")
                if it != 1:
                    nc.vector.scalar_tensor_tensor(
                        out=prod, in0=r1[:, :], scalar=0.0,
                        in1=dab[0:B1, it * QF:(it + 1) * QF],
                        op0=ALU.bypass, op1=ALU.mult)
                else:
                    r1sb = prpool.tile([B1, QF], F32, tag="r1sb",
                                       name=f"r1sb{it}")
                    nc.scalar.copy(r1sb, r1[:, :])
                    nc.gpsimd.tensor_tensor(
                        out=prod, in0=r1sb,
                        in1=dab[0:B1, it * QF:(it + 1) * QF], op=ALU.mult)
                for k in range(QF // P):
                    col = it * (QF // P) + k
                    nc.tensor.matmul(
                        risk_pm[:, col:col + 1],
                        prod[:, k * P:(k + 1) * P],
                        ones_col[0:B1, :],
                        start=True, stop=True,
                        skip_group_check=True)

            # ---------------- epilogue on [128, 16]
            r_sb = spool.tile([P, JCH], F32)
            nc.scalar.copy(r_sb, risk_pm[:, :])
            lnr = spool.tile([P, JCH], F32)
            nc.scalar.activation(out=lnr, in_=r_sb, func=AF.Ln)
            dd = spool.tile([P, JCH], F32)
            nc.vector.tensor_tensor(out=dd, in0=lnr, in1=lnw, op=ALU.subtract)
            tt = spool.tile([P, JCH], F32)
            nc.vector.scalar_tensor_tensor(out=tt, in0=dd, scalar=1.0 / N,
                                           in1=cen_cols, op0=ALU.mult,
                                           op1=ALU.mult)
            red = spool.tile([P, 1], F32)
            nc.vector.tensor_reduce(out=red, in_=tt, op=ALU.add,
                                    axis=mybir.AxisListType.X)
            fin = pfin.tile([1, 1], F32, tag="fin")
            nc.tensor.matmul(fin[:1, :], red, ones_col[:, :],
                             start=True, stop=True)
            part = spool.tile([1, 1], F32)
            nc.vector.tensor_copy(part[:1, :], fin[:1, :])
            nc.gpsimd.dma_start(out=out[:, :], in_=part[:1, :])
    return nc


_NC_CACHE = {}


def _get_nc():
    if "nc" not in _NC_CACHE:
        nc = build()
        legalize_waits(nc)
        _NC_CACHE["nc"] = nc
    return _NC_CACHE["nc"]


def _make_in_maps(survtime, censor, hazard_pred):
    s = np.ascontiguousarray(np.asarray(survtime, np.float32).reshape(-1))
    cen = np.ascontiguousarray(np.asarray(censor, np.float32).reshape(-1))
    th = np.ascontiguousarray(np.asarray(hazard_pred, np.float32).reshape(-1))
    assert s.shape == (N,) and cen.shape == (N,) and th.shape == (N,)

    p = np.arange(P, dtype=np.float32)
    ib = (p / np.float32(B1))[:, None]
    it = p[:, None]
    it2 = np.where(p < 2 * B2, np.mod(p, B2) / np.float32(SCALE2),
                   np.float32(9e9))[:, None].astype(np.float32)
    ist = np.zeros((P, 2 * B2), np.float32)
    for o in range(2 * B2):
        ist[o % B2, o] = 1.0
    pp = np.arange(P)
    S0 = (np.arange(B1)[None, :] == ((2 * pp) % B1)[:, None]).astype(np.float32)
    S1 = (np.arange(B1)[None, :] == ((2 * pp + 1) % B1)[:, None]).astype(np.float32)
    identS = np.concatenate([S0, S1], axis=1)  # [128, 2*B1]
    iota_row = np.concatenate([
        np.arange(B1, dtype=np.float32) / np.float32(B1),
        np.arange(B2, dtype=np.float32) / np.float32(SCALE2)])[None, :]

    in_maps = []
    for r in range(NCORES):
        sl = slice(r * MY, (r + 1) * MY)
        s_cm = np.ascontiguousarray(s[sl].reshape(JCH, P).T)
        th_cm = np.ascontiguousarray(th[sl].reshape(JCH, P).T)
        cen_cm = np.ascontiguousarray(cen[sl].reshape(JCH, P).T)
        pack = np.concatenate([s_cm, th_cm, cen_cm, ib, it, it2, identS, ist], axis=1)
        assert pack.shape == (P, PACKW)
        in_maps.append({
            "in_pack": np.ascontiguousarray(pack),
            "in_row": np.ascontiguousarray(s[sl][None, :]),
            "in_iota": np.ascontiguousarray(iota_row),
        })
    return in_maps


def run(survtime, censor, hazard_pred, **kw):
    in_maps = _make_in_maps(survtime, censor, hazard_pred)
    res = run_bass_kernel_spmd(_get_nc(), in_maps, list(range(NCORES)), **kw)
    total = np.float64(0.0)
    for r in range(NCORES):
        total += np.float64(np.asarray(res.results[r]["partial"]).reshape(-1)[0])
    return np.asarray(total, dtype=np.float32), res


def kernel(survtime, censor, hazard_pred):
    loss, _ = run(survtime, censor, hazard_pred)
    return loss


# revision 3
# speedup vs baseline: 1.0444x; 1.0444x over previous
"""CoxLoss (nn_CoxLoss) Trainium2 kernel v2: two-level histogram/CDF, 8-way
SPMD, AllGather-based table exchange.

risk_i = sum_{b<=a_i, t<=u_i} G2[t,b]  (2D prefix of folded 2nd differences)

  a_i = floor(s_i*B1) (level-1 bin), u_i = (s_i*B1*B2) mod B2
  Msuf[b,t] = sum_j w_j [a_j>=b][u_j>=t]  (PE matmul over j staircases,
              sharded over cores, AllGather + local reduce)
  G2 = 2nd differences over b of the t-diff of Msuf + suffix-histogram fold

Key = 2^10 bins: loss rel err ~4.9e-4 (same-key distinct-value pairs),
inside the 2e-2 gate with 40x margin. AllGather (15us fixed + 32KB) replaces
AllReduce (28.1us fixed min); the local reduce and all i-side staircases
hide under the collective.
"""
import numpy as np
import concourse.bass as bass
import concourse.mybir as mybir
from concourse.tile import TileContext
from concourse.bass_utils import run_bass_kernel_spmd

F32 = mybir.dt.float32
BF16 = mybir.dt.bfloat16
I32 = mybir.dt.int32
AF = mybir.ActivationFunctionType
ALU = mybir.AluOpType

N = 16384
P = 128
NCORES = 8
MY = N // NCORES          # 2048 rows per core
JCH = MY // P             # 16 j-chunks per core
B1 = 32                   # level-1 (value) bins: b
B2 = 32                   # level-2 bins: t
SCALE2 = float(B1 * B2)   # 2^10
QF = 512                  # i-tile width
NQ = MY // QF             # 4 i-tiles
NFOLD = P // B1           # 4 table blocks folded per matmul

# in_pack columns
C_S = 0              # s chunk-major [128,16]
C_TH = JCH           # theta
C_CEN = 2 * JCH      # censor
C_IB = 3 * JCH       # ib col (1): p/B1
C_IT = C_IB + 1      # it col (1): p
C_IT2 = C_IT + 1     # it2 col (1): p %% B2 for p<2*B2 else big
C_IDS = C_IT2 + 1    # identS2 [128,2*B1]: S_h[p,b]=[b==(2p+h)%B1]
C_IST = C_IDS + 2 * B1   # Istack [128,2*B2]: rows 0..B2-1: [o%%B2==p]
PACKW = C_IST + 2 * B2


def legalize_waits(nc, max_waits=1):
    """Insert same-engine Drains carrying excess sync waits immediately
    before each offending instruction (walrus accepts ~1 wait/instr)."""
    fn = nc.m.functions[0]
    for blk in fn.blocks:
        insts = blk.instructions
        out_list = []
        changed = False
        for ins in insts:
            si = ins.sync_info
            if si is not None and len(si.on_wait) > max_waits:
                waits = list(si.on_wait)
                keep = waits[:max_waits]
                for k, w in enumerate(waits[max_waits:]):
                    d = mybir.InstDrain(name=f"{ins.name}-w{k}", ins=[], outs=[])
                    d.engine = ins.engine
                    d.sync_info = mybir.SyncInfo(on_wait=[w], on_update=[])
                    out_list.append(d)
                si.on_wait = keep
                ins.sync_info = si
                changed = True
            out_list.append(ins)
        if changed:
            blk.instructions = out_list


def build():
    nc = bass.Bass()
    in_pack = nc.dram_tensor("in_pack", [P, PACKW], F32, kind="ExternalInput")
    in_row = nc.dram_tensor("in_row", [1, MY], F32, kind="ExternalInput")
    out = nc.dram_tensor("partial", [1, 1], F32, kind="ExternalOutput")

    msuf_dram = nc.dram_tensor("msuf_dram", [B1, B2], F32)
    msuf_sh = nc.dram_tensor("msuf_sh", [P, 2 * B2], F32,
                             addr_space="Shared")

    with TileContext(nc) as tc:
        with (
            tc.tile_pool(name="const", bufs=1) as cpool,
            tc.tile_pool(name="jstair", bufs=6) as jpool,
            tc.tile_pool(name="istair", bufs=1) as ipool,
            tc.tile_pool(name="small", bufs=1) as spool,
            tc.tile_pool(name="prod", bufs=2) as prpool,
            tc.tile_pool(name="pms", bufs=1, space="PSUM") as pms,
            tc.tile_pool(name="pr1", bufs=2, space="PSUM") as pr1,
            tc.tile_pool(name="pfin", bufs=1, space="PSUM") as pfin,
        ):
            # ---------------- input DMAs: pack on Pool; iota+s_rep on SP.
            # (HWDGE dma_start occupies the issuing engine; keep ACT free.)
            pack = cpool.tile([P, PACKW], F32)
            nc.gpsimd.dma_start(out=pack, in_=in_pack[:, :])
            iotaI = cpool.tile([P, B2], BF16)
            nc.gpsimd.iota(iotaI, [[0, 1], [1, B2]], base=0,
                           channel_multiplier=0,
                           allow_small_or_imprecise_dtypes=True)
            s_rep = cpool.tile([P, MY], F32)
            for q in range(NQ):
                nc.sync.dma_start(
                    out=s_rep[:, q * QF:(q + 1) * QF],
                    in_=in_row[:, q * QF:(q + 1) * QF].to_broadcast([P, QF]))

            s_cols = pack[:, C_S:C_S + JCH]
            th_cols = pack[:, C_TH:C_TH + JCH]
            cen_cols = pack[:, C_CEN:C_CEN + JCH]
            ib_col = pack[:, C_IB:C_IB + 1]
            it_col = pack[:, C_IT:C_IT + 1]
            it2_col = pack[:, C_IT2:C_IT2 + 1]
            identS2 = pack[:, C_IDS:C_IDS + 2 * B1]
            istack = pack[:, C_IST:C_IST + 2 * B2]

            ones_col = cpool.tile([P, 1], F32)
            nc.vector.memset(ones_col, 1.0)

            # ---------------- ACT: preload Sigmoid table on dummy data,
            # then w as soon as pack lands. Ln loads once at lnw and stays
            # resident for the epilogue lnr.
            act_t = cpool.tile([P, 2], F32)
            nc.scalar.activation(out=act_t[:, 0:1], in_=ones_col,
                                 func=AF.Sigmoid)
            w_col = cpool.tile([P, JCH], F32)
            nc.scalar.activation(out=w_col, in_=th_cols, func=AF.Sigmoid)
            lnw = cpool.tile([P, JCH], F32)
            nc.scalar.activation(out=lnw, in_=w_col, func=AF.Ln)

            # DVE touch ops (establish first-use order)
            dve_t = cpool.tile([P, 4], F32)
            nc.vector.tensor_copy(dve_t[:, 0:1], pack[:, 0:1])
            nc.vector.tensor_copy(dve_t[:, 1:2], iotaI[:, 0:1])

            # ---------------- v-space sub-bin coord: v = s - floor(s*B1)/B1
            # ([u >= t] == [v >= t/SCALE2]); floor via RNE(x-0.5) i32 convert
            def emit_v(dst, src_ap, wt, it_, ft):
                nc.vector.tensor_scalar(out=wt, in0=src_ap,
                                        scalar1=float(B1), scalar2=0.5,
                                        op0=ALU.mult, op1=ALU.subtract)
                nc.vector.tensor_copy(it_, wt)     # f32 -> i32 (RNE)
                nc.vector.tensor_copy(ft, it_)     # i32 -> f32
                nc.vector.scalar_tensor_tensor(out=dst, in0=ft,
                                               scalar=-1.0 / float(B1),
                                               in1=src_ap,
                                               op0=ALU.mult, op1=ALU.add)

            # j staircases in integer space against the on-device ramp:
            # daw = [iota <= 32s], dcw = [iota <= u]*w with u = 32*(32s - f)
            msuf_ps = pms.tile([B1, B2], F32, tag="msuf")
            uw = cpool.tile([P, JCH], F32)
            nc.vector.tensor_scalar(out=uw, in0=s_cols, scalar1=float(B1),
                                    scalar2=0.5, op0=ALU.mult,
                                    op1=ALU.subtract)
            s32_col = cpool.tile([P, JCH], F32)
            nc.vector.tensor_scalar(out=s32_col, in0=uw, scalar1=0.5,
                                    scalar2=None, op0=ALU.add)
            daws = []
            for jc in range(JCH):
                daw = jpool.tile([P, B1], BF16, tag="daw", name=f"daw{jc}")
                nc.vector.tensor_scalar(out=daw, in0=iotaI,
                                        scalar1=s32_col[:, jc:jc + 1],
                                        scalar2=None, op0=ALU.is_le)
                daws.append(daw)

            ui = cpool.tile([P, JCH], I32)
            nc.vector.tensor_copy(ui, uw)      # f32 -> i32 (RNE)
            uf = cpool.tile([P, JCH], F32)
            nc.vector.tensor_copy(uf, ui)      # i32 -> f32
            u_col = cpool.tile([P, JCH], F32)
            nc.vector.scalar_tensor_tensor(out=u_col, in0=s32_col,
                                           scalar=float(B2), in1=uf,
                                           op0=ALU.bypass, op1=ALU.subtract)
            nc.vector.tensor_scalar(out=u_col, in0=u_col, scalar1=float(B2),
                                    scalar2=None, op0=ALU.mult)

            for jc in range(JCH):
                dcw = jpool.tile([P, B2], BF16, tag="dcw", name=f"dcw{jc}")
                nc.vector.tensor_scalar(out=dcw, in0=iotaI,
                                        scalar1=u_col[:, jc:jc + 1],
                                        scalar2=w_col[:, jc:jc + 1],
                                        op0=ALU.is_le, op1=ALU.mult)
                nc.tensor.matmul(msuf_ps[:, :], daws[jc], dcw,
                                 start=(jc == 0), stop=(jc == JCH - 1))

            # Msuf -> SBUF (ACT) -> DRAM -> AllGather
            msuf_sb = spool.tile([B1, B2], F32)
            nc.scalar.copy(msuf_sb, msuf_ps[:, :])
            nc.gpsimd.dma_start(out=msuf_dram[:, :], in_=msuf_sb)
            nc.gpsimd.collective_compute(
                "AllGather", ALU.bypass,
                ins=[msuf_dram[:, :]], outs=[msuf_sh[:, :]],
                replica_groups=[list(range(NCORES))])

            # ---------------- i staircases + u_rep (DVE, hidden under AG)
            with tc.tile_wait_until(0.0055):
                uw2 = ipool.tile([P, MY], F32)
                ui2 = ipool.tile([P, MY], I32)
                uf2 = ipool.tile([P, MY], F32)
                v_rep = ipool.tile([P, MY], F32)
                emit_v(v_rep, s_rep, uw2, ui2, uf2)
                dab = ipool.tile([P, MY], F32)
                nc.vector.tensor_scalar(out=dab, in0=s_rep, scalar1=ib_col,
                                        scalar2=None, op0=ALU.is_ge)
                dct2 = ipool.tile([P, MY], BF16)
                nc.vector.tensor_scalar(out=dct2, in0=v_rep, scalar1=it2_col,
                                        scalar2=None, op0=ALU.is_ge)

            # ---------------- gathered tables: one [128, 64] load
            # (row p = gathered rows 2p | 2p+1; col-halves are separate rows)
            mt = spool.tile([P, 2 * B2], F32)
            nc.gpsimd.dma_start(out=mt, in_=msuf_sh[:, :])
            # t-diff within each col-half
            yacc = spool.tile([P, 2 * B2], F32)
            nc.vector.tensor_copy(yacc[:, 0:1], mt[:, 0:1])
            nc.vector.tensor_tensor(out=yacc[:, 1:B2], in0=mt[:, 1:B2],
                                    in1=mt[:, 0:B2 - 1], op=ALU.subtract)
            nc.vector.tensor_copy(yacc[:, B2:B2 + 1], mt[:, B2:B2 + 1])
            nc.vector.tensor_tensor(out=yacc[:, B2 + 1:2 * B2],
                                    in0=mt[:, B2 + 1:2 * B2],
                                    in1=mt[:, B2:2 * B2 - 1], op=ALU.subtract)

            # fold all 8 blocks + transpose in 2 accumulating matmuls:
            # YT[t,b] = sum_h sum_p yacc[p, h*B2+t] * S_h[p, b]
            yt_ps = pms.tile([B2, B1], F32, tag="ytps")
            nc.tensor.matmul(yt_ps[:, :], yacc[:, 0:B2], identS2[:, 0:B1],
                             start=True, stop=False)
            nc.tensor.matmul(yt_ps[:, :], yacc[:, B2:2 * B2],
                             identS2[:, B1:2 * B1], start=False, stop=True)
            ytp = spool.tile([B2, B1 + 2], F32)
            nc.vector.tensor_copy(ytp[:, 1:B1 + 1], yt_ps[:, :])
            nc.vector.tensor_copy(ytp[:, 0:1], yt_ps[:, 0:1])  # left pad
            nc.vector.memset(ytp[:, B1 + 1:B1 + 2], 0.0)       # right pad

            # G2 = 2*P1 - P2 - P0  [B2, B1]
            g2 = spool.tile([B2, B1], F32)
            nc.vector.scalar_tensor_tensor(out=g2, in0=ytp[:, 1:B1 + 1],
                                           scalar=2.0, in1=ytp[:, 2:B1 + 2],
                                           op0=ALU.mult, op1=ALU.subtract)
            nc.vector.tensor_tensor(out=g2, in0=g2, in1=ytp[:, 0:B1],
                                    op=ALU.subtract)
            # hh fold on t=0 row: H[b] = YT[0, b] = ytp[0, 1+b]
            hh = spool.tile([1, B1], F32)
            nc.vector.tensor_tensor(out=hh[:1, :], in0=ytp[0:1, 2:B1 + 2],
                                    in1=ytp[0:1, 1:B1 + 1], op=ALU.subtract)
            nc.vector.tensor_tensor(out=g2[0:1, :], in0=g2[0:1, :],
                                    in1=hh[:1, :], op=ALU.add)
            nc.vector.tensor_tensor(out=g2[0:1, 0:1], in0=g2[0:1, 0:1],
                                    in1=ytp[0:1, 1:2], op=ALU.add)

            # duplicate g2 rows into [2*B2, B1] (PE), then bf16 hi/lo:
            # rows 0..B2-1 = bf16(g2), rows B2.. = g2 - bf16(g2). One r1
            # matmul per i-tile contracts hi+lo together over 2*B2 rows.
            g2d_ps = pms.tile([2 * B2, B1], F32, tag="g2dps")
            nc.tensor.matmul(g2d_ps[:, :], istack[0:B2, :], g2,
                             start=True, stop=True)
            g2s = spool.tile([2 * B2, B1], BF16)
            nc.vector.tensor_copy(g2s, g2d_ps[:, :])
            nc.vector.tensor_tensor(out=g2s[B2:2 * B2, :],
                                    in0=g2d_ps[B2:2 * B2, :],
                                    in1=g2s[B2:2 * B2, :], op=ALU.subtract)

            # ---------------- R1 + prod + partition reduce -> risk_pm
            risk_pm = pfin.tile([P, JCH], F32, tag="riskpm")
            for it in range(NQ):
                r1 = pr1.tile([B1, QF], F32, tag="r1", name=f"r1_{it}")
                nc.tensor.matmul(r1[:, :], g2s,
                                 dct2[0:2 * B2, it * QF:(it + 1) * QF],
                                 start=True, stop=True)
                prod = prpool.tile([B1, QF], F32, tag="prod",
                                   name=f"prod{it}")
                if it != 1:
                    nc.vector.scalar_tensor_tensor(
                        out=prod, in0=r1[:, :], scalar=0.0,
                        in1=dab[0:B1, it * QF:(it + 1) * QF],
                        op0=ALU.bypass, op1=ALU.mult)
                else:
                    r1sb = prpool.tile([B1, QF], F32, tag="r1sb",
                                       name=f"r1sb{it}")
                    nc.scalar.copy(r1sb, r1[:, :])
                    nc.gpsimd.tensor_tensor(
                        out=prod, in0=r1sb,
                        in1=dab[0:B1, it * QF:(it + 1) * QF], op=ALU.mult)
                for k in range(QF // P):
                    col = it * (QF // P) + k
                    nc.tensor.matmul(
                        risk_pm[:, col:col + 1],
                        prod[:, k * P:(k + 1) * P],
                        ones_col[0:B1, :],
                        start=True, stop=True,
                        skip_group_check=True)

            # ---------------- epilogue on [128, 16]
            r_sb = spool.tile([P, JCH], F32)
            nc.scalar.copy(r_sb, risk_pm[:, :])
            lnr = spool.tile([P, JCH], F32)
            nc.scalar.activation(out=lnr, in_=r_sb, func=AF.Ln)
            dd = spool.tile([P, JCH], F32)
            nc.vector.tensor_tensor(out=dd, in0=lnr, in1=lnw, op=ALU.subtract)
            tt = spool.tile([P, JCH], F32)
            nc.vector.scalar_tensor_tensor(out=tt, in0=dd, scalar=1.0 / N,
                                           in1=cen_cols, op0=ALU.mult,
                                           op1=ALU.mult)
            red = spool.tile([P, 1], F32)
            nc.vector.tensor_reduce(out=red, in_=tt, op=ALU.add,
                                    axis=mybir.AxisListType.X)
            fin = pfin.tile([1, 1], F32, tag="fin")
            nc.tensor.matmul(fin[:1, :], red, ones_col[:, :],
                             start=True, stop=True)
            part = spool.tile([1, 1], F32)
            nc.vector.tensor_copy(part[:1, :], fin[:1, :])
            nc.gpsimd.dma_start(out=out[:, :], in_=part[:1, :])
    return nc


_NC_CACHE = {}


def _get_nc():
    if "nc" not in _NC_CACHE:
        nc = build()
        legalize_waits(nc)
        _NC_CACHE["nc"] = nc
    return _NC_CACHE["nc"]


def _make_in_maps(survtime, censor, hazard_pred):
    s = np.ascontiguousarray(np.asarray(survtime, np.float32).reshape(-1))
    cen = np.ascontiguousarray(np.asarray(censor, np.float32).reshape(-1))
    th = np.ascontiguousarray(np.asarray(hazard_pred, np.float32).reshape(-1))
    assert s.shape == (N,) and cen.shape == (N,) and th.shape == (N,)

    p = np.arange(P, dtype=np.float32)
    ib = (p / np.float32(B1))[:, None]
    it = p[:, None]
    it2 = np.where(p < 2 * B2, np.mod(p, B2) / np.float32(SCALE2),
                   np.float32(9e9))[:, None].astype(np.float32)
    ist = np.zeros((P, 2 * B2), np.float32)
    for o in range(2 * B2):
        ist[o % B2, o] = 1.0
    pp = np.arange(P)
    S0 = (np.arange(B1)[None, :] == ((2 * pp) % B1)[:, None]).astype(np.float32)
    S1 = (np.arange(B1)[None, :] == ((2 * pp + 1) % B1)[:, None]).astype(np.float32)
    identS = np.concatenate([S0, S1], axis=1)  # [128, 2*B1]

    in_maps = []
    for r in range(NCORES):
        sl = slice(r * MY, (r + 1) * MY)
        s_cm = np.ascontiguousarray(s[sl].reshape(JCH, P).T)
        th_cm = np.ascontiguousarray(th[sl].reshape(JCH, P).T)
        cen_cm = np.ascontiguousarray(cen[sl].reshape(JCH, P).T)
        pack = np.concatenate([s_cm, th_cm, cen_cm, ib, it, it2, identS, ist], axis=1)
        assert pack.shape == (P, PACKW)
        in_maps.append({
            "in_pack": np.ascontiguousarray(pack),
            "in_row": np.ascontiguousarray(s[sl][None, :]),
        })
    return in_maps


def run(survtime, censor, hazard_pred, **kw):
    in_maps = _make_in_maps(survtime, censor, hazard_pred)
    res = run_bass_kernel_spmd(_get_nc(), in_maps, list(range(NCORES)), **kw)
    total = np.float64(0.0)
    for r in range(NCORES):
        total += np.float64(np.asarray(res.results[r]["partial"]).reshape(-1)[0])
    return np.asarray(total, dtype=np.float32), res


def kernel(survtime, censor, hazard_pred):
    loss, _ = run(survtime, censor, hazard_pred)
    return loss


# revision 4
# speedup vs baseline: 1.0476x; 1.0031x over previous
"""CoxLoss (nn_CoxLoss) Trainium2 kernel v2: two-level histogram/CDF, 8-way
SPMD, AllGather-based table exchange.

risk_i = sum_{b<=a_i, t<=u_i} G2[t,b]  (2D prefix of folded 2nd differences)

  a_i = floor(s_i*B1) (level-1 bin), u_i = (s_i*B1*B2) mod B2
  Msuf[b,t] = sum_j w_j [a_j>=b][u_j>=t]  (PE matmul over j staircases,
              sharded over cores, AllGather + local reduce)
  G2 = 2nd differences over b of the t-diff of Msuf + suffix-histogram fold

Key = 2^10 bins: loss rel err ~4.9e-4 (same-key distinct-value pairs),
inside the 2e-2 gate with 40x margin. AllGather (15us fixed + 32KB) replaces
AllReduce (28.1us fixed min); the local reduce and all i-side staircases
hide under the collective.
"""
import numpy as np
import concourse.bass as bass
import concourse.mybir as mybir
from concourse.tile import TileContext
from concourse.bass_utils import run_bass_kernel_spmd

F32 = mybir.dt.float32
BF16 = mybir.dt.bfloat16
I32 = mybir.dt.int32
AF = mybir.ActivationFunctionType
ALU = mybir.AluOpType

N = 16384
P = 128
NCORES = 8
MY = N // NCORES          # 2048 rows per core
JCH = MY // P             # 16 j-chunks per core
B1 = 16                   # level-1 (value) bins: b
B2 = 64                   # level-2 bins: t
SCALE2 = float(B1 * B2)   # 2^10
QF = 512                  # i-tile width
NQ = MY // QF             # 4 i-tiles
NFOLD = P // B1           # 4 table blocks folded per matmul

# in_pack columns
C_S = 0              # s chunk-major [128,16]
C_TH = JCH           # theta
C_CEN = 2 * JCH      # censor
C_IB = 3 * JCH       # ib col (1): p/B1
C_IT = C_IB + 1      # it col (1): p
C_IT2 = C_IT + 1     # it2 col (1): (p %% B2)/SCALE2
C_IDS = C_IT2 + 1    # identS [128,B1]: [b == p %% B1]
C_IST = C_IDS + B1   # Istack [rows 0..B2-1, 2*B2]: [o %% B2 == p]
PACKW = C_IST + 2 * B2


def legalize_waits(nc, max_waits=1):
    """Insert same-engine Drains carrying excess sync waits immediately
    before each offending instruction (walrus accepts ~1 wait/instr)."""
    fn = nc.m.functions[0]
    for blk in fn.blocks:
        insts = blk.instructions
        out_list = []
        changed = False
        for ins in insts:
            si = ins.sync_info
            if si is not None and len(si.on_wait) > max_waits:
                waits = list(si.on_wait)
                keep = waits[:max_waits]
                for k, w in enumerate(waits[max_waits:]):
                    d = mybir.InstDrain(name=f"{ins.name}-w{k}", ins=[], outs=[])
                    d.engine = ins.engine
                    d.sync_info = mybir.SyncInfo(on_wait=[w], on_update=[])
                    out_list.append(d)
                si.on_wait = keep
                ins.sync_info = si
                changed = True
            out_list.append(ins)
        if changed:
            blk.instructions = out_list


def build():
    nc = bass.Bass()
    in_pack = nc.dram_tensor("in_pack", [P, PACKW], F32, kind="ExternalInput")
    in_row = nc.dram_tensor("in_row", [1, MY], F32, kind="ExternalInput")
    out = nc.dram_tensor("partial", [1, 1], F32, kind="ExternalOutput")

    msuf_dram = nc.dram_tensor("msuf_dram", [B1, B2], F32)
    msuf_sh = nc.dram_tensor("msuf_sh", [P, B2], F32,
                             addr_space="Shared")

    with TileContext(nc) as tc:
        with (
            tc.tile_pool(name="const", bufs=1) as cpool,
            tc.tile_pool(name="jstair", bufs=6) as jpool,
            tc.tile_pool(name="istair", bufs=1) as ipool,
            tc.tile_pool(name="small", bufs=1) as spool,
            tc.tile_pool(name="prod", bufs=2) as prpool,
            tc.tile_pool(name="pms", bufs=1, space="PSUM") as pms,
            tc.tile_pool(name="pr1", bufs=2, space="PSUM") as pr1,
            tc.tile_pool(name="pfin", bufs=1, space="PSUM") as pfin,
        ):
            # ---------------- input DMAs: pack on Pool; iota+s_rep on SP.
            # (HWDGE dma_start occupies the issuing engine; keep ACT free.)
            pack = cpool.tile([P, PACKW], F32)
            nc.gpsimd.dma_start(out=pack, in_=in_pack[:, :])
            iotaI = cpool.tile([P, B2], BF16)
            nc.gpsimd.iota(iotaI, [[0, 1], [1, B2]], base=0,
                           channel_multiplier=0,
                           allow_small_or_imprecise_dtypes=True)
            s_rep = cpool.tile([P, MY], F32)
            for q in range(NQ):
                nc.sync.dma_start(
                    out=s_rep[:, q * QF:(q + 1) * QF],
                    in_=in_row[:, q * QF:(q + 1) * QF].to_broadcast([P, QF]))

            s_cols = pack[:, C_S:C_S + JCH]
            th_cols = pack[:, C_TH:C_TH + JCH]
            cen_cols = pack[:, C_CEN:C_CEN + JCH]
            ib_col = pack[:, C_IB:C_IB + 1]
            it_col = pack[:, C_IT:C_IT + 1]
            it2_col = pack[:, C_IT2:C_IT2 + 1]
            identS = pack[:, C_IDS:C_IDS + B1]
            istack = pack[:, C_IST:C_IST + 2 * B2]

            ones_col = cpool.tile([P, 1], F32)
            nc.vector.memset(ones_col, 1.0)

            # ---------------- ACT: preload Sigmoid table on dummy data,
            # then w as soon as pack lands. Ln loads once at lnw and stays
            # resident for the epilogue lnr.
            act_t = cpool.tile([P, 2], F32)
            nc.scalar.activation(out=act_t[:, 0:1], in_=ones_col,
                                 func=AF.Sigmoid)
            w_col = cpool.tile([P, JCH], F32)
            nc.scalar.activation(out=w_col, in_=th_cols, func=AF.Sigmoid)
            lnw = cpool.tile([P, JCH], F32)
            nc.scalar.activation(out=lnw, in_=w_col, func=AF.Ln)

            # DVE touch ops (establish first-use order)
            dve_t = cpool.tile([P, 4], F32)
            nc.vector.tensor_copy(dve_t[:, 0:1], pack[:, 0:1])
            nc.vector.tensor_copy(dve_t[:, 1:2], iotaI[:, 0:1])

            # ---------------- v-space sub-bin coord: v = s - floor(s*B1)/B1
            # ([u >= t] == [v >= t/SCALE2]); floor via RNE(x-0.5) i32 convert
            def emit_v(dst, src_ap, wt, it_, ft):
                nc.vector.tensor_scalar(out=wt, in0=src_ap,
                                        scalar1=float(B1), scalar2=0.5,
                                        op0=ALU.mult, op1=ALU.subtract)
                nc.vector.tensor_copy(it_, wt)     # f32 -> i32 (RNE)
                nc.vector.tensor_copy(ft, it_)     # i32 -> f32
                nc.vector.scalar_tensor_tensor(out=dst, in0=ft,
                                               scalar=-1.0 / float(B1),
                                               in1=src_ap,
                                               op0=ALU.mult, op1=ALU.add)

            # j staircases in integer space against the on-device ramp:
            # daw = [iota <= 32s], dcw = [iota <= u]*w with u = 32*(32s - f)
            msuf_ps = pms.tile([B1, B2], F32, tag="msuf")
            uw = cpool.tile([P, JCH], F32)
            nc.vector.tensor_scalar(out=uw, in0=s_cols, scalar1=float(B1),
                                    scalar2=0.5, op0=ALU.mult,
                                    op1=ALU.subtract)
            s32_col = cpool.tile([P, JCH], F32)
            nc.vector.tensor_scalar(out=s32_col, in0=uw, scalar1=0.5,
                                    scalar2=None, op0=ALU.add)
            daws = []
            for jc in range(JCH):
                daw = jpool.tile([P, B1], BF16, tag="daw", name=f"daw{jc}")
                nc.vector.tensor_scalar(out=daw, in0=iotaI[:, 0:B1],
                                        scalar1=s32_col[:, jc:jc + 1],
                                        scalar2=None, op0=ALU.is_le)
                daws.append(daw)

            ui = cpool.tile([P, JCH], I32)
            nc.vector.tensor_copy(ui, uw)      # f32 -> i32 (RNE)
            uf = cpool.tile([P, JCH], F32)
            nc.vector.tensor_copy(uf, ui)      # i32 -> f32
            u_col = cpool.tile([P, JCH], F32)
            nc.vector.scalar_tensor_tensor(out=u_col, in0=s32_col,
                                           scalar=float(B2), in1=uf,
                                           op0=ALU.bypass, op1=ALU.subtract)
            nc.vector.tensor_scalar(out=u_col, in0=u_col, scalar1=float(B2),
                                    scalar2=None, op0=ALU.mult)

            for jc in range(JCH):
                dcw = jpool.tile([P, B2], BF16, tag="dcw", name=f"dcw{jc}")
                nc.vector.tensor_scalar(out=dcw, in0=iotaI,
                                        scalar1=u_col[:, jc:jc + 1],
                                        scalar2=w_col[:, jc:jc + 1],
                                        op0=ALU.is_le, op1=ALU.mult)
                nc.tensor.matmul(msuf_ps[:, :], daws[jc], dcw,
                                 start=(jc == 0), stop=(jc == JCH - 1))

            # Msuf -> SBUF (ACT) -> DRAM -> AllGather
            msuf_sb = spool.tile([B1, B2], F32)
            nc.scalar.copy(msuf_sb, msuf_ps[:, :])
            nc.gpsimd.dma_start(out=msuf_dram[:, :], in_=msuf_sb)
            nc.gpsimd.collective_compute(
                "AllGather", ALU.bypass,
                ins=[msuf_dram[:, :]], outs=[msuf_sh[:, :]],
                replica_groups=[list(range(NCORES))])

            # ---------------- i staircases + u_rep (DVE, hidden under AG)
            with tc.tile_wait_until(0.0055):
                uw2 = ipool.tile([P, MY], F32)
                ui2 = ipool.tile([P, MY], I32)
                uf2 = ipool.tile([P, MY], F32)
                v_rep = ipool.tile([P, MY], F32)
                emit_v(v_rep, s_rep, uw2, ui2, uf2)
                dab = ipool.tile([P, MY], F32)
                nc.vector.tensor_scalar(out=dab, in0=s_rep, scalar1=ib_col,
                                        scalar2=None, op0=ALU.is_ge)
                dct2 = ipool.tile([P, MY], BF16)
                nc.vector.tensor_scalar(out=dct2, in0=v_rep, scalar1=it2_col,
                                        scalar2=None, op0=ALU.is_ge)

            # ---------------- gathered tables: one [128, 64] load
            # (row p = core p//B1's table row p%B1 -- partitions exactly full)
            mt = spool.tile([P, B2], F32)
            nc.gpsimd.dma_start(out=mt, in_=msuf_sh[:, :])
            yacc = spool.tile([P, B2], F32)
            nc.vector.tensor_copy(yacc[:, 0:1], mt[:, 0:1])
            nc.vector.tensor_tensor(out=yacc[:, 1:B2], in0=mt[:, 1:B2],
                                    in1=mt[:, 0:B2 - 1], op=ALU.subtract)

            # fold all 8 blocks + transpose in ONE matmul:
            # YT[t,b] = sum_p yacc[p, t] * [b == p % B1]
            yt_ps = pms.tile([B2, B1], F32, tag="ytps")
            nc.tensor.matmul(yt_ps[:, :], yacc, identS, start=True, stop=True)
            ytp = spool.tile([B2, B1 + 2], F32)
            nc.vector.tensor_copy(ytp[:, 1:B1 + 1], yt_ps[:, :])
            nc.vector.tensor_copy(ytp[:, 0:1], yt_ps[:, 0:1])  # left pad
            nc.vector.memset(ytp[:, B1 + 1:B1 + 2], 0.0)       # right pad

            # G2 = 2*P1 - P2 - P0  [B2, B1]
            g2 = spool.tile([B2, B1], F32)
            nc.vector.scalar_tensor_tensor(out=g2, in0=ytp[:, 1:B1 + 1],
                                           scalar=2.0, in1=ytp[:, 2:B1 + 2],
                                           op0=ALU.mult, op1=ALU.subtract)
            nc.vector.tensor_tensor(out=g2, in0=g2, in1=ytp[:, 0:B1],
                                    op=ALU.subtract)
            # hh fold on t=0 row: H[b] = YT[0, b] = ytp[0, 1+b]
            hh = spool.tile([1, B1], F32)
            nc.vector.tensor_tensor(out=hh[:1, :], in0=ytp[0:1, 2:B1 + 2],
                                    in1=ytp[0:1, 1:B1 + 1], op=ALU.subtract)
            nc.vector.tensor_tensor(out=g2[0:1, :], in0=g2[0:1, :],
                                    in1=hh[:1, :], op=ALU.add)
            nc.vector.tensor_tensor(out=g2[0:1, 0:1], in0=g2[0:1, 0:1],
                                    in1=ytp[0:1, 1:2], op=ALU.add)

            # duplicate g2 rows into [2*B2, B1] (PE), then bf16 hi/lo:
            # rows 0..B2-1 = bf16(g2), rows B2.. = g2 - bf16(g2). One r1
            # matmul per i-tile contracts hi+lo together over 2*B2 rows.
            g2d_ps = pms.tile([2 * B2, B1], F32, tag="g2dps")
            nc.tensor.matmul(g2d_ps[:, :], istack[0:B2, :], g2,
                             start=True, stop=True)
            g2s = spool.tile([2 * B2, B1], BF16)
            nc.vector.tensor_copy(g2s, g2d_ps[:, :])
            nc.vector.tensor_tensor(out=g2s[B2:2 * B2, :],
                                    in0=g2d_ps[B2:2 * B2, :],
                                    in1=g2s[B2:2 * B2, :], op=ALU.subtract)

            # ---------------- R1 + prod + partition reduce -> risk_pm
            risk_pm = pfin.tile([P, JCH], F32, tag="riskpm")
            for it in range(NQ):
                r1 = pr1.tile([B1, QF], F32, tag="r1", name=f"r1_{it}")
                nc.tensor.matmul(r1[:, :], g2s,
                                 dct2[0:2 * B2, it * QF:(it + 1) * QF],
                                 start=True, stop=True)
                prod = prpool.tile([B1, QF], F32, tag="prod",
                                   name=f"prod{it}")
                if it != 1:
                    nc.vector.scalar_tensor_tensor(
                        out=prod, in0=r1[:, :], scalar=0.0,
                        in1=dab[0:B1, it * QF:(it + 1) * QF],
                        op0=ALU.bypass, op1=ALU.mult)
                else:
                    r1sb = prpool.tile([B1, QF], F32, tag="r1sb",
                                       name=f"r1sb{it}")
                    nc.scalar.copy(r1sb, r1[:, :])
                    nc.gpsimd.tensor_tensor(
                        out=prod, in0=r1sb,
                        in1=dab[0:B1, it * QF:(it + 1) * QF], op=ALU.mult)
                for k in range(QF // P):
                    col = it * (QF // P) + k
                    nc.tensor.matmul(
                        risk_pm[:, col:col + 1],
                        prod[:, k * P:(k + 1) * P],
                        ones_col[0:B1, :],
                        start=True, stop=True,
                        skip_group_check=True)

            # ---------------- epilogue on [128, 16]
            r_sb = spool.tile([P, JCH], F32)
            nc.scalar.copy(r_sb, risk_pm[:, :])
            lnr = spool.tile([P, JCH], F32)
            nc.scalar.activation(out=lnr, in_=r_sb, func=AF.Ln)
            dd = spool.tile([P, JCH], F32)
            nc.vector.tensor_tensor(out=dd, in0=lnr, in1=lnw, op=ALU.subtract)
            tt = spool.tile([P, JCH], F32)
            nc.vector.scalar_tensor_tensor(out=tt, in0=dd, scalar=1.0 / N,
                                           in1=cen_cols, op0=ALU.mult,
                                           op1=ALU.mult)
            red = spool.tile([P, 1], F32)
            nc.vector.tensor_reduce(out=red, in_=tt, op=ALU.add,
                                    axis=mybir.AxisListType.X)
            fin = pfin.tile([1, 1], F32, tag="fin")
            nc.tensor.matmul(fin[:1, :], red, ones_col[:, :],
                             start=True, stop=True)
            part = spool.tile([1, 1], F32)
            nc.vector.tensor_copy(part[:1, :], fin[:1, :])
            nc.gpsimd.dma_start(out=out[:, :], in_=part[:1, :])
    return nc


_NC_CACHE = {}


def _get_nc():
    if "nc" not in _NC_CACHE:
        nc = build()
        legalize_waits(nc)
        _NC_CACHE["nc"] = nc
    return _NC_CACHE["nc"]


def _make_in_maps(survtime, censor, hazard_pred):
    s = np.ascontiguousarray(np.asarray(survtime, np.float32).reshape(-1))
    cen = np.ascontiguousarray(np.asarray(censor, np.float32).reshape(-1))
    th = np.ascontiguousarray(np.asarray(hazard_pred, np.float32).reshape(-1))
    assert s.shape == (N,) and cen.shape == (N,) and th.shape == (N,)

    p = np.arange(P, dtype=np.float32)
    ib = (p / np.float32(B1))[:, None]
    it = p[:, None]
    it2 = (np.mod(p, B2) / np.float32(SCALE2))[:, None].astype(np.float32)
    ist = np.zeros((P, 2 * B2), np.float32)
    for o in range(2 * B2):
        ist[o % B2, o] = 1.0
    pp = np.arange(P)
    identS = (np.arange(B1)[None, :] == (pp % B1)[:, None]).astype(np.float32)

    in_maps = []
    for r in range(NCORES):
        sl = slice(r * MY, (r + 1) * MY)
        s_cm = np.ascontiguousarray(s[sl].reshape(JCH, P).T)
        th_cm = np.ascontiguousarray(th[sl].reshape(JCH, P).T)
        cen_cm = np.ascontiguousarray(cen[sl].reshape(JCH, P).T)
        pack = np.concatenate([s_cm, th_cm, cen_cm, ib, it, it2, identS, ist], axis=1)
        assert pack.shape == (P, PACKW)
        in_maps.append({
            "in_pack": np.ascontiguousarray(pack),
            "in_row": np.ascontiguousarray(s[sl][None, :]),
        })
    return in_maps


def run(survtime, censor, hazard_pred, **kw):
    in_maps = _make_in_maps(survtime, censor, hazard_pred)
    res = run_bass_kernel_spmd(_get_nc(), in_maps, list(range(NCORES)), **kw)
    total = np.float64(0.0)
    for r in range(NCORES):
        total += np.float64(np.asarray(res.results[r]["partial"]).reshape(-1)[0])
    return np.asarray(total, dtype=np.float32), res


def kernel(survtime, censor, hazard_pred):
    loss, _ = run(survtime, censor, hazard_pred)
    return loss


# revision 5
# speedup vs baseline: 1.0624x; 1.0141x over previous
"""CoxLoss (nn_CoxLoss) Trainium2 kernel v2: two-level histogram/CDF, 8-way
SPMD, AllGather-based table exchange.

risk_i = sum_{b<=a_i, t<=u_i} G2[t,b]  (2D prefix of folded 2nd differences)

  a_i = floor(s_i*B1) (level-1 bin), u_i = (s_i*B1*B2) mod B2
  Msuf[b,t] = sum_j w_j [a_j>=b][u_j>=t]  (PE matmul over j staircases,
              sharded over cores, AllGather + local reduce)
  G2 = 2nd differences over b of the t-diff of Msuf + suffix-histogram fold

Key = 2^10 bins: loss rel err ~4.9e-4 (same-key distinct-value pairs),
inside the 2e-2 gate with 40x margin. AllGather (15us fixed + 32KB) replaces
AllReduce (28.1us fixed min); the local reduce and all i-side staircases
hide under the collective.
"""
import numpy as np
import concourse.bass as bass
import concourse.mybir as mybir
from concourse.tile import TileContext
from concourse.bass_utils import run_bass_kernel_spmd

F32 = mybir.dt.float32
BF16 = mybir.dt.bfloat16
I32 = mybir.dt.int32
AF = mybir.ActivationFunctionType
ALU = mybir.AluOpType

N = 16384
P = 128
NCORES = 8
MY = N // NCORES          # 2048 rows per core
JCH = MY // P             # 16 j-chunks per core
B1 = 16                   # level-1 (value) bins: b
B2 = 64                   # level-2 bins: t
SCALE2 = float(B1 * B2)   # 2^10
QF = 512                  # i-tile width
NQ = MY // QF             # 4 i-tiles
NFOLD = P // B1           # 4 table blocks folded per matmul

# in_pack columns
C_S = 0              # s chunk-major [128,16]
C_TH = JCH           # theta
C_CEN = 2 * JCH      # censor
C_IB = 3 * JCH       # ib col (1): p/B1
C_IT = C_IB + 1      # it col (1): p
C_IT2 = C_IT + 1     # it2 col (1): (p %% B2)/SCALE2
C_IDS = C_IT2 + 1    # identS [128,B1]: [b == p %% B1]
C_IST = C_IDS + B1   # Istack [rows 0..B2-1, 2*B2]: [o %% B2 == p]
PACKW = C_IST + 2 * B2


def legalize_waits(nc, max_waits=1):
    """Insert same-engine Drains carrying excess sync waits immediately
    before each offending instruction (walrus accepts ~1 wait/instr)."""
    fn = nc.m.functions[0]
    for blk in fn.blocks:
        insts = blk.instructions
        out_list = []
        changed = False
        for ins in insts:
            si = ins.sync_info
            if si is not None and len(si.on_wait) > max_waits:
                waits = list(si.on_wait)
                keep = waits[:max_waits]
                for k, w in enumerate(waits[max_waits:]):
                    d = mybir.InstDrain(name=f"{ins.name}-w{k}", ins=[], outs=[])
                    d.engine = ins.engine
                    d.sync_info = mybir.SyncInfo(on_wait=[w], on_update=[])
                    out_list.append(d)
                si.on_wait = keep
                ins.sync_info = si
                changed = True
            out_list.append(ins)
        if changed:
            blk.instructions = out_list


def build():
    nc = bass.Bass()
    in_pack = nc.dram_tensor("in_pack", [P, PACKW], F32, kind="ExternalInput")
    in_row = nc.dram_tensor("in_row", [1, MY], F32, kind="ExternalInput")
    out = nc.dram_tensor("partial", [1, 1], F32, kind="ExternalOutput")

    msuf_dram = nc.dram_tensor("msuf_dram", [B1, B2], F32)
    msuf_sh = nc.dram_tensor("msuf_sh", [P, B2], F32,
                             addr_space="Shared")

    with TileContext(nc) as tc:
        with (
            tc.tile_pool(name="const", bufs=1) as cpool,
            tc.tile_pool(name="jstair", bufs=6) as jpool,
            tc.tile_pool(name="istair", bufs=1) as ipool,
            tc.tile_pool(name="small", bufs=1) as spool,
            tc.tile_pool(name="prod", bufs=4) as prpool,
            tc.tile_pool(name="pms", bufs=1, space="PSUM") as pms,
            tc.tile_pool(name="pr1", bufs=3, space="PSUM") as pr1,
            tc.tile_pool(name="pfin", bufs=1, space="PSUM") as pfin,
        ):
            # ---------------- input DMAs: pack on Pool; iota+s_rep on SP.
            # (HWDGE dma_start occupies the issuing engine; keep ACT free.)
            pack = cpool.tile([P, PACKW], F32)
            nc.gpsimd.dma_start(out=pack, in_=in_pack[:, :])
            iotaI = cpool.tile([P, B2], BF16)
            nc.gpsimd.iota(iotaI, [[0, 1], [1, B2]], base=0,
                           channel_multiplier=0,
                           allow_small_or_imprecise_dtypes=True)
            s_rep = cpool.tile([P, MY], F32)
            for q in range(NQ):
                nc.sync.dma_start(
                    out=s_rep[:, q * QF:(q + 1) * QF],
                    in_=in_row[:, q * QF:(q + 1) * QF].to_broadcast([P, QF]))

            s_cols = pack[:, C_S:C_S + JCH]
            th_cols = pack[:, C_TH:C_TH + JCH]
            cen_cols = pack[:, C_CEN:C_CEN + JCH]
            ib_col = pack[:, C_IB:C_IB + 1]
            it_col = pack[:, C_IT:C_IT + 1]
            it2_col = pack[:, C_IT2:C_IT2 + 1]
            identS = pack[:, C_IDS:C_IDS + B1]
            istack = pack[:, C_IST:C_IST + 2 * B2]

            ones_col = cpool.tile([P, 1], F32)
            nc.vector.memset(ones_col, 1.0)

            # ---------------- ACT: preload Sigmoid table on dummy data,
            # then w as soon as pack lands. Ln loads once at lnw and stays
            # resident for the epilogue lnr.
            act_t = cpool.tile([P, 2], F32)
            nc.scalar.activation(out=act_t[:, 0:1], in_=ones_col,
                                 func=AF.Sigmoid)
            w_col = cpool.tile([P, JCH], F32)
            nc.scalar.activation(out=w_col, in_=th_cols, func=AF.Sigmoid)
            lnw = cpool.tile([P, JCH], F32)
            nc.scalar.activation(out=lnw, in_=w_col, func=AF.Ln)

            # DVE touch ops (establish first-use order)
            dve_t = cpool.tile([P, 4], F32)
            nc.vector.tensor_copy(dve_t[:, 0:1], pack[:, 0:1])
            nc.vector.tensor_copy(dve_t[:, 1:2], iotaI[:, 0:1])

            # ---------------- v-space sub-bin coord: v = s - floor(s*B1)/B1
            # ([u >= t] == [v >= t/SCALE2]); floor via RNE(x-0.5) i32 convert
            def emit_v(dst, src_ap, wt, it_, ft):
                nc.vector.tensor_scalar(out=wt, in0=src_ap,
                                        scalar1=float(B1), scalar2=0.5,
                                        op0=ALU.mult, op1=ALU.subtract)
                nc.vector.tensor_copy(it_, wt)     # f32 -> i32 (RNE)
                nc.vector.tensor_copy(ft, it_)     # i32 -> f32
                nc.vector.scalar_tensor_tensor(out=dst, in0=ft,
                                               scalar=-1.0 / float(B1),
                                               in1=src_ap,
                                               op0=ALU.mult, op1=ALU.add)

            # j staircases in integer space against the on-device ramp:
            # daw = [iota <= 32s], dcw = [iota <= u]*w with u = 32*(32s - f)
            msuf_ps = pms.tile([B1, B2], F32, tag="msuf")
            uw = cpool.tile([P, JCH], F32)
            nc.vector.tensor_scalar(out=uw, in0=s_cols, scalar1=float(B1),
                                    scalar2=0.5, op0=ALU.mult,
                                    op1=ALU.subtract)
            s32_col = cpool.tile([P, JCH], F32)
            nc.vector.tensor_scalar(out=s32_col, in0=uw, scalar1=0.5,
                                    scalar2=None, op0=ALU.add)
            daws = []
            for jc in range(JCH):
                daw = jpool.tile([P, B1], BF16, tag="daw", name=f"daw{jc}")
                nc.vector.tensor_scalar(out=daw, in0=iotaI[:, 0:B1],
                                        scalar1=s32_col[:, jc:jc + 1],
                                        scalar2=None, op0=ALU.is_le)
                daws.append(daw)

            ui = cpool.tile([P, JCH], I32)
            nc.vector.tensor_copy(ui, uw)      # f32 -> i32 (RNE)
            uf = cpool.tile([P, JCH], F32)
            nc.vector.tensor_copy(uf, ui)      # i32 -> f32
            u_col = cpool.tile([P, JCH], F32)
            nc.vector.scalar_tensor_tensor(out=u_col, in0=s32_col,
                                           scalar=float(B2), in1=uf,
                                           op0=ALU.bypass, op1=ALU.subtract)
            nc.vector.tensor_scalar(out=u_col, in0=u_col, scalar1=float(B2),
                                    scalar2=None, op0=ALU.mult)

            for jc in range(JCH):
                dcw = jpool.tile([P, B2], BF16, tag="dcw", name=f"dcw{jc}")
                nc.vector.tensor_scalar(out=dcw, in0=iotaI,
                                        scalar1=u_col[:, jc:jc + 1],
                                        scalar2=w_col[:, jc:jc + 1],
                                        op0=ALU.is_le, op1=ALU.mult)
                nc.tensor.matmul(msuf_ps[:, :], daws[jc], dcw,
                                 start=(jc == 0), stop=(jc == JCH - 1))

            # Msuf -> SBUF (ACT) -> DRAM -> AllGather
            msuf_sb = spool.tile([B1, B2], F32)
            nc.scalar.copy(msuf_sb, msuf_ps[:, :])
            nc.gpsimd.dma_start(out=msuf_dram[:, :], in_=msuf_sb)
            nc.gpsimd.collective_compute(
                "AllGather", ALU.bypass,
                ins=[msuf_dram[:, :]], outs=[msuf_sh[:, :]],
                replica_groups=[list(range(NCORES))])

            # ---------------- i staircases + u_rep (DVE, hidden under AG)
            with tc.tile_wait_until(0.0055):
                uw2 = ipool.tile([P, MY], F32)
                ui2 = ipool.tile([P, MY], I32)
                uf2 = ipool.tile([P, MY], F32)
                v_rep = ipool.tile([P, MY], F32)
                emit_v(v_rep, s_rep, uw2, ui2, uf2)
                dab = ipool.tile([P, MY], F32)
                nc.vector.tensor_scalar(out=dab, in0=s_rep, scalar1=ib_col,
                                        scalar2=None, op0=ALU.is_ge)
                dct2 = ipool.tile([P, MY], BF16)
                nc.vector.tensor_scalar(out=dct2, in0=v_rep, scalar1=it2_col,
                                        scalar2=None, op0=ALU.is_ge)

            # ---------------- gathered tables: one [128, 64] load
            # (row p = core p//B1's table row p%B1 -- partitions exactly full)
            mt = spool.tile([P, B2], F32)
            nc.gpsimd.dma_start(out=mt, in_=msuf_sh[:, :])
            yacc = spool.tile([P, B2], F32)
            nc.vector.tensor_copy(yacc[:, 0:1], mt[:, 0:1])
            nc.vector.tensor_tensor(out=yacc[:, 1:B2], in0=mt[:, 1:B2],
                                    in1=mt[:, 0:B2 - 1], op=ALU.subtract)

            # fold all 8 blocks + transpose in ONE matmul:
            # YT[t,b] = sum_p yacc[p, t] * [b == p % B1]
            yt_ps = pms.tile([B2, B1], F32, tag="ytps")
            nc.tensor.matmul(yt_ps[:, :], yacc, identS, start=True, stop=True)
            ytp = spool.tile([B2, B1 + 2], F32)
            nc.vector.tensor_copy(ytp[:, 1:B1 + 1], yt_ps[:, :])
            nc.vector.tensor_copy(ytp[:, 0:1], yt_ps[:, 0:1])  # left pad
            nc.vector.memset(ytp[:, B1 + 1:B1 + 2], 0.0)       # right pad

            # G2 = 2*P1 - P2 - P0  [B2, B1]
            g2 = spool.tile([B2, B1], F32)
            nc.vector.scalar_tensor_tensor(out=g2, in0=ytp[:, 1:B1 + 1],
                                           scalar=2.0, in1=ytp[:, 2:B1 + 2],
                                           op0=ALU.mult, op1=ALU.subtract)
            nc.vector.tensor_tensor(out=g2, in0=g2, in1=ytp[:, 0:B1],
                                    op=ALU.subtract)
            # hh fold on t=0 row: H[b] = YT[0, b] = ytp[0, 1+b]
            hh = spool.tile([1, B1], F32)
            nc.vector.tensor_tensor(out=hh[:1, :], in0=ytp[0:1, 2:B1 + 2],
                                    in1=ytp[0:1, 1:B1 + 1], op=ALU.subtract)
            nc.vector.tensor_tensor(out=g2[0:1, :], in0=g2[0:1, :],
                                    in1=hh[:1, :], op=ALU.add)
            nc.vector.tensor_tensor(out=g2[0:1, 0:1], in0=g2[0:1, 0:1],
                                    in1=ytp[0:1, 1:2], op=ALU.add)

            # duplicate g2 rows into [2*B2, B1] (PE), then bf16 hi/lo:
            # rows 0..B2-1 = bf16(g2), rows B2.. = g2 - bf16(g2). One r1
            # matmul per i-tile contracts hi+lo together over 2*B2 rows.
            g2d_ps = pms.tile([2 * B2, B1], F32, tag="g2dps")
            nc.tensor.matmul(g2d_ps[:, :], istack[0:B2, :], g2,
                             start=True, stop=True)
            g2s = spool.tile([2 * B2, B1], BF16)
            nc.vector.tensor_copy(g2s, g2d_ps[:, :])
            nc.vector.tensor_tensor(out=g2s[B2:2 * B2, :],
                                    in0=g2d_ps[B2:2 * B2, :],
                                    in1=g2s[B2:2 * B2, :], op=ALU.subtract)

            # ---------------- R1 + prod + partition reduce -> risk_pm
            risk_pm = pfin.tile([P, JCH], F32, tag="riskpm")
            for it in range(NQ):
                r1 = pr1.tile([B1, QF], F32, tag="r1", name=f"r1_{it}")
                nc.tensor.matmul(r1[:, :], g2s,
                                 dct2[0:2 * B2, it * QF:(it + 1) * QF],
                                 start=True, stop=True)
                prod = prpool.tile([B1, QF], F32, tag="prod",
                                   name=f"prod{it}")
                if it != 1:
                    nc.vector.scalar_tensor_tensor(
                        out=prod, in0=r1[:, :], scalar=0.0,
                        in1=dab[0:B1, it * QF:(it + 1) * QF],
                        op0=ALU.bypass, op1=ALU.mult)
                else:
                    r1sb = prpool.tile([B1, QF], F32, tag="r1sb",
                                       name=f"r1sb{it}")
                    nc.scalar.copy(r1sb, r1[:, :])
                    nc.gpsimd.tensor_tensor(
                        out=prod, in0=r1sb,
                        in1=dab[0:B1, it * QF:(it + 1) * QF], op=ALU.mult)
                for k in range(QF // P):
                    col = it * (QF // P) + k
                    nc.tensor.matmul(
                        risk_pm[:, col:col + 1],
                        prod[:, k * P:(k + 1) * P],
                        ones_col[0:B1, :],
                        start=True, stop=True,
                        skip_group_check=True)

            # ---------------- epilogue on [128, 16]
            r_sb = spool.tile([P, JCH], F32)
            nc.scalar.copy(r_sb, risk_pm[:, :])
            lnr = spool.tile([P, JCH], F32)
            nc.scalar.activation(out=lnr, in_=r_sb, func=AF.Ln)
            dd = spool.tile([P, JCH], F32)
            nc.vector.tensor_tensor(out=dd, in0=lnr, in1=lnw, op=ALU.subtract)
            tt = spool.tile([P, JCH], F32)
            nc.vector.scalar_tensor_tensor(out=tt, in0=dd, scalar=1.0 / N,
                                           in1=cen_cols, op0=ALU.mult,
                                           op1=ALU.mult)
            red = spool.tile([P, 1], F32)
            nc.vector.tensor_reduce(out=red, in_=tt, op=ALU.add,
                                    axis=mybir.AxisListType.X)
            fin = pfin.tile([1, 1], F32, tag="fin")
            nc.tensor.matmul(fin[:1, :], red, ones_col[:, :],
                             start=True, stop=True)
            part = spool.tile([1, 1], F32)
            nc.vector.tensor_copy(part[:1, :], fin[:1, :])
            nc.gpsimd.dma_start(out=out[:, :], in_=part[:1, :])
    return nc


_NC_CACHE = {}


def _get_nc():
    if "nc" not in _NC_CACHE:
        nc = build()
        legalize_waits(nc)
        _NC_CACHE["nc"] = nc
    return _NC_CACHE["nc"]


def _make_in_maps(survtime, censor, hazard_pred):
    s = np.ascontiguousarray(np.asarray(survtime, np.float32).reshape(-1))
    cen = np.ascontiguousarray(np.asarray(censor, np.float32).reshape(-1))
    th = np.ascontiguousarray(np.asarray(hazard_pred, np.float32).reshape(-1))
    assert s.shape == (N,) and cen.shape == (N,) and th.shape == (N,)

    p = np.arange(P, dtype=np.float32)
    ib = (p / np.float32(B1))[:, None]
    it = p[:, None]
    it2 = (np.mod(p, B2) / np.float32(SCALE2))[:, None].astype(np.float32)
    ist = np.zeros((P, 2 * B2), np.float32)
    for o in range(2 * B2):
        ist[o % B2, o] = 1.0
    pp = np.arange(P)
    identS = (np.arange(B1)[None, :] == (pp % B1)[:, None]).astype(np.float32)

    in_maps = []
    for r in range(NCORES):
        sl = slice(r * MY, (r + 1) * MY)
        s_cm = np.ascontiguousarray(s[sl].reshape(JCH, P).T)
        th_cm = np.ascontiguousarray(th[sl].reshape(JCH, P).T)
        cen_cm = np.ascontiguousarray(cen[sl].reshape(JCH, P).T)
        pack = np.concatenate([s_cm, th_cm, cen_cm, ib, it, it2, identS, ist], axis=1)
        assert pack.shape == (P, PACKW)
        in_maps.append({
            "in_pack": np.ascontiguousarray(pack),
            "in_row": np.ascontiguousarray(s[sl][None, :]),
        })
    return in_maps


def run(survtime, censor, hazard_pred, **kw):
    in_maps = _make_in_maps(survtime, censor, hazard_pred)
    res = run_bass_kernel_spmd(_get_nc(), in_maps, list(range(NCORES)), **kw)
    total = np.float64(0.0)
    for r in range(NCORES):
        total += np.float64(np.asarray(res.results[r]["partial"]).reshape(-1)[0])
    return np.asarray(total, dtype=np.float32), res


def kernel(survtime, censor, hazard_pred):
    loss, _ = run(survtime, censor, hazard_pred)
    return loss


# revision 6
# speedup vs baseline: 1.0767x; 1.0135x over previous
"""CoxLoss (nn_CoxLoss) Trainium2 kernel v2: two-level histogram/CDF, 8-way
SPMD, AllGather-based table exchange.

risk_i = sum_{b<=a_i, t<=u_i} G2[t,b]  (2D prefix of folded 2nd differences)

  a_i = floor(s_i*B1) (level-1 bin), u_i = (s_i*B1*B2) mod B2
  Msuf[b,t] = sum_j w_j [a_j>=b][u_j>=t]  (PE matmul over j staircases,
              sharded over cores, AllGather + local reduce)
  G2 = 2nd differences over b of the t-diff of Msuf + suffix-histogram fold

Key = 2^10 bins: loss rel err ~4.9e-4 (same-key distinct-value pairs),
inside the 2e-2 gate with 40x margin. AllGather (15us fixed + 32KB) replaces
AllReduce (28.1us fixed min); the local reduce and all i-side staircases
hide under the collective.
"""
import numpy as np
import concourse.bass as bass
import concourse.mybir as mybir
from concourse.tile import TileContext
from concourse.bass_utils import run_bass_kernel_spmd

F32 = mybir.dt.float32
BF16 = mybir.dt.bfloat16
I32 = mybir.dt.int32
AF = mybir.ActivationFunctionType
ALU = mybir.AluOpType

N = 16384
P = 128
NCORES = 8
MY = N // NCORES          # 2048 rows per core
JCH = MY // P             # 16 j-chunks per core
B1 = 16                   # level-1 (value) bins: b
B2 = 64                   # level-2 bins: t
SCALE2 = float(B1 * B2)   # 2^10
QF = 512                  # i-tile width
NQ = MY // QF             # 4 i-tiles
NFOLD = P // B1           # 4 table blocks folded per matmul

# in_pack columns
C_S = 0              # s chunk-major [128,16]
C_TH = JCH           # theta
C_CEN = 2 * JCH      # censor
C_IB = 3 * JCH       # ib col (1): p/B1
C_IT = C_IB + 1      # it col (1): p
C_IT2 = C_IT + 1     # it2 col (1): (p %% B2)/SCALE2
C_IDS = C_IT2 + 1    # identS [128,B1]: [b == p %% B1]
C_IST = C_IDS + B1   # Istack [rows 0..B2-1, 2*B2]: [o %% B2 == p]
PACKW = C_IST + 2 * B2


def legalize_waits(nc, max_waits=1):
    """Insert same-engine Drains carrying excess sync waits immediately
    before each offending instruction (walrus accepts ~1 wait/instr)."""
    fn = nc.m.functions[0]
    for blk in fn.blocks:
        insts = blk.instructions
        out_list = []
        changed = False
        for ins in insts:
            si = ins.sync_info
            if si is not None and len(si.on_wait) > max_waits:
                waits = list(si.on_wait)
                keep = waits[:max_waits]
                for k, w in enumerate(waits[max_waits:]):
                    d = mybir.InstDrain(name=f"{ins.name}-w{k}", ins=[], outs=[])
                    d.engine = ins.engine
                    d.sync_info = mybir.SyncInfo(on_wait=[w], on_update=[])
                    out_list.append(d)
                si.on_wait = keep
                ins.sync_info = si
                changed = True
            out_list.append(ins)
        if changed:
            blk.instructions = out_list


def build():
    nc = bass.Bass()
    in_pack = nc.dram_tensor("in_pack", [P, PACKW], F32, kind="ExternalInput")
    in_row = nc.dram_tensor("in_row", [1, MY], F32, kind="ExternalInput")
    out = nc.dram_tensor("partial", [1, 1], F32, kind="ExternalOutput")

    msuf_dram = nc.dram_tensor("msuf_dram", [B1, B2], F32)
    msuf_sh = nc.dram_tensor("msuf_sh", [P, B2], F32,
                             addr_space="Shared")

    with TileContext(nc) as tc:
        with (
            tc.tile_pool(name="const", bufs=1) as cpool,
            tc.tile_pool(name="jstair", bufs=16) as jpool,
            tc.tile_pool(name="istair", bufs=1) as ipool,
            tc.tile_pool(name="small", bufs=1) as spool,
            tc.tile_pool(name="prod", bufs=4) as prpool,
            tc.tile_pool(name="pms", bufs=1, space="PSUM") as pms,
            tc.tile_pool(name="pr1", bufs=3, space="PSUM") as pr1,
            tc.tile_pool(name="pfin", bufs=1, space="PSUM") as pfin,
        ):
            # ---------------- input DMAs: pack on Pool; iota+s_rep on SP.
            # (HWDGE dma_start occupies the issuing engine; keep ACT free.)
            pack = cpool.tile([P, PACKW], F32)
            nc.gpsimd.dma_start(out=pack, in_=in_pack[:, :])
            iotaI = cpool.tile([P, B2], BF16)
            nc.gpsimd.iota(iotaI, [[0, 1], [1, B2]], base=0,
                           channel_multiplier=0,
                           allow_small_or_imprecise_dtypes=True)
            s_rep = cpool.tile([P, MY], F32)
            for q in range(NQ):
                nc.sync.dma_start(
                    out=s_rep[:, q * QF:(q + 1) * QF],
                    in_=in_row[:, q * QF:(q + 1) * QF].to_broadcast([P, QF]))

            s_cols = pack[:, C_S:C_S + JCH]
            th_cols = pack[:, C_TH:C_TH + JCH]
            cen_cols = pack[:, C_CEN:C_CEN + JCH]
            ib_col = pack[:, C_IB:C_IB + 1]
            it_col = pack[:, C_IT:C_IT + 1]
            it2_col = pack[:, C_IT2:C_IT2 + 1]
            identS = pack[:, C_IDS:C_IDS + B1]
            istack = pack[:, C_IST:C_IST + 2 * B2]

            ones_col = cpool.tile([P, 1], F32)
            nc.vector.memset(ones_col, 1.0)

            # ---------------- ACT: preload Sigmoid table on dummy data,
            # then w as soon as pack lands. Ln loads once at lnw and stays
            # resident for the epilogue lnr.
            act_t = cpool.tile([P, 2], F32)
            nc.scalar.activation(out=act_t[:, 0:1], in_=ones_col,
                                 func=AF.Sigmoid)
            w_col = cpool.tile([P, JCH], F32)
            nc.scalar.activation(out=w_col, in_=th_cols, func=AF.Sigmoid)
            lnw = cpool.tile([P, JCH], F32)
            nc.scalar.activation(out=lnw, in_=w_col, func=AF.Ln)

            # DVE touch ops (establish first-use order)
            dve_t = cpool.tile([P, 4], F32)
            nc.vector.tensor_copy(dve_t[:, 0:1], pack[:, 0:1])
            nc.vector.tensor_copy(dve_t[:, 1:2], iotaI[:, 0:1])

            # ---------------- v-space sub-bin coord: v = s - floor(s*B1)/B1
            # ([u >= t] == [v >= t/SCALE2]); floor via RNE(x-0.5) i32 convert
            def emit_v(dst, src_ap, wt, it_, ft):
                nc.vector.tensor_scalar(out=wt, in0=src_ap,
                                        scalar1=float(B1), scalar2=0.5,
                                        op0=ALU.mult, op1=ALU.subtract)
                nc.vector.tensor_copy(it_, wt)     # f32 -> i32 (RNE)
                nc.vector.tensor_copy(ft, it_)     # i32 -> f32
                nc.vector.scalar_tensor_tensor(out=dst, in0=ft,
                                               scalar=-1.0 / float(B1),
                                               in1=src_ap,
                                               op0=ALU.mult, op1=ALU.add)

            # j staircases in integer space against the on-device ramp:
            # daw = [iota <= 32s], dcw = [iota <= u]*w with u = 32*(32s - f)
            msuf_ps = pms.tile([B1, B2], F32, tag="msuf")
            uw = cpool.tile([P, JCH], F32)
            nc.vector.tensor_scalar(out=uw, in0=s_cols, scalar1=float(B1),
                                    scalar2=0.5, op0=ALU.mult,
                                    op1=ALU.subtract)
            s32_col = cpool.tile([P, JCH], F32)
            nc.vector.tensor_scalar(out=s32_col, in0=uw, scalar1=0.5,
                                    scalar2=None, op0=ALU.add)
            daws = []
            for jc in range(JCH):
                daw = jpool.tile([P, B1], BF16, tag="daw", name=f"daw{jc}")
                nc.vector.tensor_scalar(out=daw, in0=iotaI[:, 0:B1],
                                        scalar1=s32_col[:, jc:jc + 1],
                                        scalar2=None, op0=ALU.is_le)
                daws.append(daw)

            ui = cpool.tile([P, JCH], I32)
            nc.vector.tensor_copy(ui, uw)      # f32 -> i32 (RNE)
            uf = cpool.tile([P, JCH], F32)
            nc.vector.tensor_copy(uf, ui)      # i32 -> f32
            u_col = cpool.tile([P, JCH], F32)
            nc.vector.scalar_tensor_tensor(out=u_col, in0=s32_col,
                                           scalar=float(B2), in1=uf,
                                           op0=ALU.bypass, op1=ALU.subtract)
            nc.vector.tensor_scalar(out=u_col, in0=u_col, scalar1=float(B2),
                                    scalar2=None, op0=ALU.mult)

            for jc in range(JCH):
                dcw = jpool.tile([P, B2], BF16, tag="dcw", name=f"dcw{jc}")
                nc.vector.tensor_scalar(out=dcw, in0=iotaI,
                                        scalar1=u_col[:, jc:jc + 1],
                                        scalar2=w_col[:, jc:jc + 1],
                                        op0=ALU.is_le, op1=ALU.mult)
                nc.tensor.matmul(msuf_ps[:, :], daws[jc], dcw,
                                 start=(jc == 0), stop=(jc == JCH - 1))

            # Msuf -> SBUF (ACT) -> DRAM -> AllGather
            msuf_sb = spool.tile([B1, B2], F32)
            nc.scalar.copy(msuf_sb, msuf_ps[:, :])
            nc.gpsimd.dma_start(out=msuf_dram[:, :], in_=msuf_sb)
            nc.gpsimd.collective_compute(
                "AllGather", ALU.bypass,
                ins=[msuf_dram[:, :]], outs=[msuf_sh[:, :]],
                replica_groups=[list(range(NCORES))])

            # ---------------- i staircases + u_rep (DVE, hidden under AG)
            with tc.tile_wait_until(0.0055):
                uw2 = ipool.tile([P, MY], F32)
                ui2 = ipool.tile([P, MY], I32)
                uf2 = ipool.tile([P, MY], F32)
                v_rep = ipool.tile([P, MY], F32)
                emit_v(v_rep, s_rep, uw2, ui2, uf2)
                dab = ipool.tile([P, MY], F32)
                nc.vector.tensor_scalar(out=dab, in0=s_rep, scalar1=ib_col,
                                        scalar2=None, op0=ALU.is_ge)
                dct2 = ipool.tile([P, MY], BF16)
                nc.vector.tensor_scalar(out=dct2, in0=v_rep, scalar1=it2_col,
                                        scalar2=None, op0=ALU.is_ge)

            # ---------------- gathered tables: one [128, 64] load
            # (row p = core p//B1's table row p%B1 -- partitions exactly full)
            mt = spool.tile([P, B2], F32)
            nc.gpsimd.dma_start(out=mt, in_=msuf_sh[:, :])
            yacc = spool.tile([P, B2], F32)
            nc.vector.tensor_copy(yacc[:, 0:1], mt[:, 0:1])
            nc.vector.tensor_tensor(out=yacc[:, 1:B2], in0=mt[:, 1:B2],
                                    in1=mt[:, 0:B2 - 1], op=ALU.subtract)

            # fold all 8 blocks + transpose in ONE matmul:
            # YT[t,b] = sum_p yacc[p, t] * [b == p % B1]
            yt_ps = pms.tile([B2, B1], F32, tag="ytps")
            nc.tensor.matmul(yt_ps[:, :], yacc, identS, start=True, stop=True)
            ytp = spool.tile([B2, B1 + 2], F32)
            nc.vector.tensor_copy(ytp[:, 1:B1 + 1], yt_ps[:, :])
            nc.vector.tensor_copy(ytp[:, 0:1], yt_ps[:, 0:1])  # left pad
            nc.vector.memset(ytp[:, B1 + 1:B1 + 2], 0.0)       # right pad

            # G2 = 2*P1 - P2 - P0  [B2, B1]
            g2 = spool.tile([B2, B1], F32)
            nc.vector.scalar_tensor_tensor(out=g2, in0=ytp[:, 1:B1 + 1],
                                           scalar=2.0, in1=ytp[:, 2:B1 + 2],
                                           op0=ALU.mult, op1=ALU.subtract)
            nc.vector.tensor_tensor(out=g2, in0=g2, in1=ytp[:, 0:B1],
                                    op=ALU.subtract)
            # hh fold on t=0 row: H[b] = YT[0, b] = ytp[0, 1+b]
            hh = spool.tile([1, B1], F32)
            nc.vector.tensor_tensor(out=hh[:1, :], in0=ytp[0:1, 2:B1 + 2],
                                    in1=ytp[0:1, 1:B1 + 1], op=ALU.subtract)
            nc.vector.tensor_tensor(out=g2[0:1, :], in0=g2[0:1, :],
                                    in1=hh[:1, :], op=ALU.add)
            nc.vector.tensor_tensor(out=g2[0:1, 0:1], in0=g2[0:1, 0:1],
                                    in1=ytp[0:1, 1:2], op=ALU.add)

            # duplicate g2 rows into [2*B2, B1] (PE), then bf16 hi/lo:
            # rows 0..B2-1 = bf16(g2), rows B2.. = g2 - bf16(g2). One r1
            # matmul per i-tile contracts hi+lo together over 2*B2 rows.
            g2d_ps = pms.tile([2 * B2, B1], F32, tag="g2dps")
            nc.tensor.matmul(g2d_ps[:, :], istack[0:B2, :], g2,
                             start=True, stop=True)
            g2s = spool.tile([2 * B2, B1], BF16)
            nc.vector.tensor_copy(g2s, g2d_ps[:, :])
            nc.vector.tensor_tensor(out=g2s[B2:2 * B2, :],
                                    in0=g2d_ps[B2:2 * B2, :],
                                    in1=g2s[B2:2 * B2, :], op=ALU.subtract)

            # ---------------- R1 + prod + partition reduce -> risk_pm
            risk_pm = pfin.tile([P, JCH], F32, tag="riskpm")
            for it in range(NQ):
                r1 = pr1.tile([B1, QF], F32, tag="r1", name=f"r1_{it}")
                nc.tensor.matmul(r1[:, :], g2s,
                                 dct2[0:2 * B2, it * QF:(it + 1) * QF],
                                 start=True, stop=True)
                prod = prpool.tile([B1, QF], F32, tag="prod",
                                   name=f"prod{it}")
                if it != 1:
                    nc.vector.scalar_tensor_tensor(
                        out=prod, in0=r1[:, :], scalar=0.0,
                        in1=dab[0:B1, it * QF:(it + 1) * QF],
                        op0=ALU.bypass, op1=ALU.mult)
                else:
                    r1sb = prpool.tile([B1, QF], F32, tag="r1sb",
                                       name=f"r1sb{it}")
                    nc.scalar.copy(r1sb, r1[:, :])
                    nc.gpsimd.tensor_tensor(
                        out=prod, in0=r1sb,
                        in1=dab[0:B1, it * QF:(it + 1) * QF], op=ALU.mult)
                for k in range(QF // P):
                    col = it * (QF // P) + k
                    nc.tensor.matmul(
                        risk_pm[:, col:col + 1],
                        prod[:, k * P:(k + 1) * P],
                        ones_col[0:B1, :],
                        start=True, stop=True,
                        skip_group_check=True)

            # ---------------- epilogue on [128, 16]
            r_sb = spool.tile([P, JCH], F32)
            nc.scalar.copy(r_sb, risk_pm[:, :])
            lnr = spool.tile([P, JCH], F32)
            nc.scalar.activation(out=lnr, in_=r_sb, func=AF.Ln)
            dd = spool.tile([P, JCH], F32)
            nc.vector.tensor_tensor(out=dd, in0=lnr, in1=lnw, op=ALU.subtract)
            tt = spool.tile([P, JCH], F32)
            nc.vector.scalar_tensor_tensor(out=tt, in0=dd, scalar=1.0 / N,
                                           in1=cen_cols, op0=ALU.mult,
                                           op1=ALU.mult)
            red = spool.tile([P, 1], F32)
            nc.vector.tensor_reduce(out=red, in_=tt, op=ALU.add,
                                    axis=mybir.AxisListType.X)
            fin = pfin.tile([1, 1], F32, tag="fin")
            nc.tensor.matmul(fin[:1, :], red, ones_col[:, :],
                             start=True, stop=True)
            part = spool.tile([1, 1], F32)
            nc.vector.tensor_copy(part[:1, :], fin[:1, :])
            nc.gpsimd.dma_start(out=out[:, :], in_=part[:1, :])
    return nc


_NC_CACHE = {}


def _get_nc():
    if "nc" not in _NC_CACHE:
        nc = build()
        legalize_waits(nc)
        _NC_CACHE["nc"] = nc
    return _NC_CACHE["nc"]


def _make_in_maps(survtime, censor, hazard_pred):
    s = np.ascontiguousarray(np.asarray(survtime, np.float32).reshape(-1))
    cen = np.ascontiguousarray(np.asarray(censor, np.float32).reshape(-1))
    th = np.ascontiguousarray(np.asarray(hazard_pred, np.float32).reshape(-1))
    assert s.shape == (N,) and cen.shape == (N,) and th.shape == (N,)

    p = np.arange(P, dtype=np.float32)
    ib = (p / np.float32(B1))[:, None]
    it = p[:, None]
    it2 = (np.mod(p, B2) / np.float32(SCALE2))[:, None].astype(np.float32)
    ist = np.zeros((P, 2 * B2), np.float32)
    for o in range(2 * B2):
        ist[o % B2, o] = 1.0
    pp = np.arange(P)
    identS = (np.arange(B1)[None, :] == (pp % B1)[:, None]).astype(np.float32)

    in_maps = []
    for r in range(NCORES):
        sl = slice(r * MY, (r + 1) * MY)
        s_cm = np.ascontiguousarray(s[sl].reshape(JCH, P).T)
        th_cm = np.ascontiguousarray(th[sl].reshape(JCH, P).T)
        cen_cm = np.ascontiguousarray(cen[sl].reshape(JCH, P).T)
        pack = np.concatenate([s_cm, th_cm, cen_cm, ib, it, it2, identS, ist], axis=1)
        assert pack.shape == (P, PACKW)
        in_maps.append({
            "in_pack": np.ascontiguousarray(pack),
            "in_row": np.ascontiguousarray(s[sl][None, :]),
        })
    return in_maps


def run(survtime, censor, hazard_pred, **kw):
    in_maps = _make_in_maps(survtime, censor, hazard_pred)
    res = run_bass_kernel_spmd(_get_nc(), in_maps, list(range(NCORES)), **kw)
    total = np.float64(0.0)
    for r in range(NCORES):
        total += np.float64(np.asarray(res.results[r]["partial"]).reshape(-1)[0])
    return np.asarray(total, dtype=np.float32), res


def kernel(survtime, censor, hazard_pred):
    loss, _ = run(survtime, censor, hazard_pred)
    return loss
